# revision 3
# baseline (speedup 1.0000x reference)
"""Trainium2 Bass kernel for 2-layer GCN + 2-step propagation + log_softmax.

Strategy (8 NeuronCores, SPMD), v2 — indirect-DMA gather SpMM:
  - Nodes row-sharded: core c owns srcs [12500c, 12500(c+1)).
  - Within a core, srcs are degree-sorted (host permutation); rank r lives at
    (block j = r//128, partition p = r%128).  Each 128-src block j gets a
    uniform slot budget W_j = 8*ceil(max_deg_in_block/8); a src's edges plus
    zero-padding fill its W_j slots.  Padding slots point at a table row that
    is exactly zero (a zero-degree pad node's logits).
  - MLP (x @ W1 -> relu -> @ W2) in bf16 on the tensor engine.  x arrives
    host-transposed (xT), so no PE transposes are needed; output is
    node-major logits [12544, 16] f32 -> DRAM -> AllGather -> full table
    [100352, 16].
  - SpMM per iteration: one indirect_dma_start per chunk of blocks gathers
    table rows for every slot (HW DGE, 64B/row) into [128, slots, 16]; DVE
    tree-adds reduce each src's W_j slots to agg[p, 16]; fixup
    scale*agg + alpha*L on DVE.  Iteration 1 writes node-major logits to
    DRAM for the second AllGather; iteration 2 keeps results in SBUF.
  - log_softmax batched over the whole shard (2 ACT table loads total).
  - Host inverse-permutes rows of the result.
"""

import sys

sys.path.insert(0, "/opt/trn_rl_repo")

import numpy as np
import ml_dtypes

_COMPILED_CACHE = {}

N_NODES = 100000
N_FEAT = 512
HIDDEN = 128
N_CLASS = 16
N_EDGES = 3200000
ALPHA = 0.25
N_ITERS = 2

NCORES = 8
SHARD = N_NODES // NCORES          # 12500
SHARD_PAD = 12544                  # multiple of 128
P = 128
NT = SHARD_PAD // P                # 98 blocks of 128 srcs
ZR = 12543                         # global table row that is always zero
CHUNK_SLOTS = 512                  # max gathered slots per chunk


def _host_prep(edge_src, edge_dst):
    """Degree-sort permutation, slot budgets, gather index arrays."""
    es = np.asarray(edge_src).astype(np.int64)
    ed = np.asarray(edge_dst).astype(np.int64)

    deg_full = np.bincount(es, minlength=N_NODES).astype(np.int64)

    perms = []          # per core: rank -> local id (len SHARD_PAD)
    deg_rank = []       # per core: degree by rank
    grank = np.empty(N_NODES, dtype=np.int64)   # node id -> global table row
    for c in range(NCORES):
        d = deg_full[c * SHARD:(c + 1) * SHARD]
        perm = np.argsort(-d, kind="stable")            # ranks 0..12499
        dr = np.concatenate([d[perm], np.zeros(SHARD_PAD - SHARD, np.int64)])
        perm_pad = np.concatenate(
            [perm, np.arange(SHARD, SHARD_PAD)])        # pads at the end
        rank_of = np.empty(SHARD, dtype=np.int64)
        rank_of[perm] = np.arange(SHARD)
        grank[c * SHARD:(c + 1) * SHARD] = c * SHARD_PAD + rank_of
        perms.append(perm_pad)
        deg_rank.append(dr)

    # shared slot budget per block: max degree in block across all cores
    W = np.empty(NT, dtype=np.int64)
    for j in range(NT):
        m = max(int(dr[j * P:(j + 1) * P].max()) for dr in deg_rank)
        W[j] = max(8, -(-m // 8) * 8)
    off = np.concatenate([[0], np.cumsum(W)])
    slots_pp = int(off[-1])

    # chunks of consecutive blocks, <= CHUNK_SLOTS slots each
    chunks = []         # (j0, nblocks, slot_off, slot_cnt)
    j0 = 0
    while j0 < NT:
        j1 = j0 + 1
        while j1 < NT and off[j1 + 1] - off[j0] <= CHUNK_SLOTS:
            j1 += 1
        chunks.append((j0, j1 - j0, int(off[j0]), int(off[j1] - off[j0])))
        j0 = j1

    # per-core gather index arrays [128, slots_pp] int32
    gdst = grank[ed]
    idx_arrays = []
    for c in range(NCORES):
        m = es // SHARD == c
        s_loc = es[m] - c * SHARD
        r = grank[es[m]] - c * SHARD_PAD        # rank of src
        g = gdst[m]
        order = np.argsort(r, kind="stable")
        r = r[order]
        g = g[order]
        # within-src slot position
        cnt = np.bincount(r, minlength=SHARD_PAD)
        start = np.concatenate([[0], np.cumsum(cnt)])[:-1]
        k = np.arange(len(r)) - start[r]
        col = off[r // P] + k
        row = r % P
        idx = np.full((P, slots_pp), ZR, dtype=np.int32)
        idx[row, col] = g
        idx_arrays.append(idx)

    return deg_full, perms, W, chunks, slots_pp, idx_arrays


def _build_program(W, chunks, slots_pp):
    import concourse.bass as bass
    import concourse.tile as tile
    import concourse.mybir as mybir
    from concourse import bacc

    f32 = mybir.dt.float32
    bf16 = mybir.dt.bfloat16
    i32 = mybir.dt.int32

    off = np.concatenate([[0], np.cumsum(W)])

    nc = bacc.Bacc("TRN2", target_bir_lowering=False, debug=False,
                   num_devices=NCORES)

    # ---- I/O ----
    xT_in = nc.dram_tensor("xT", [N_FEAT, SHARD_PAD], bf16,
                           kind="ExternalInput").ap()
    w1_in = nc.dram_tensor("w1p", [P, N_FEAT], bf16,
                           kind="ExternalInput").ap()
    w2_in = nc.dram_tensor("w2", [HIDDEN, N_CLASS], bf16,
                           kind="ExternalInput").ap()
    scale_in = nc.dram_tensor("scale16", [P, NT * N_CLASS], f32,
                              kind="ExternalInput").ap()
    idx_in = nc.dram_tensor("gidx", [P, slots_pp], i32,
                            kind="ExternalInput").ap()
    out_ext = nc.dram_tensor("out", [SHARD_PAD, N_CLASS], f32,
                             kind="ExternalOutput").ap()

    # ---- internal DRAM ----
    lt_dram = nc.dram_tensor("lt", [SHARD_PAD, N_CLASS], f32)
    new_dram = nc.dram_tensor("newlog", [SHARD_PAD, N_CLASS], f32)
    tbl_dram = [
        nc.dram_tensor(f"tbl{i}", [NCORES * SHARD_PAD, N_CLASS], f32,
                       addr_space="Shared")
        for i in range(2)
    ]

    FC = NT * N_CLASS  # 1568

    with tile.TileContext(nc) as tc:
        with (
            tc.tile_pool(name="persist", bufs=1) as pp,
            tc.tile_pool(name="mlp", bufs=3) as mp,
            tc.tile_pool(name="mlp_ps", bufs=4, space="PSUM") as mps,
            tc.tile_pool(name="gat", bufs=2) as gp,
            tc.tile_pool(name="stage", bufs=2) as sp,
        ):
            # ---------- persistent loads ----------
            w1_sb = pp.tile([P, N_FEAT], bf16)
            nc.sync.dma_start(w1_sb[:, :], w1_in)
            w2_sb = pp.tile([HIDDEN, N_CLASS], bf16)
            nc.sync.dma_start(w2_sb[:, :], w2_in)
            scale_sb = pp.tile([P, FC], f32)
            nc.sync.dma_start(scale_sb[:, :], scale_in)
            gidx_sb = pp.tile([P, slots_pp], i32)
            nc.sync.dma_start(gidx_sb[:, :], idx_in)
            L_sb = pp.tile([P, FC], f32)
            res_sb = pp.tile([P, FC], f32)

            # ---------- MLP ----------
            for t in range(NT):
                xt = mp.tile([P, N_FEAT], bf16, tag="xt")
                for k in range(4):
                    nc.sync.dma_start(
                        xt[:, k * P:(k + 1) * P],
                        xT_in[k * P:(k + 1) * P, t * P:(t + 1) * P])
                hps = mps.tile([P, P], f32, tag="hps")
                for k in range(4):
                    nc.tensor.matmul(out=hps[:, :],
                                     lhsT=w1_sb[:, k * P:(k + 1) * P],
                                     rhs=xt[:, k * P:(k + 1) * P],
                                     start=(k == 0), stop=(k == 3))
                h_sb = mp.tile([P, P], bf16, tag="h_sb")
                nc.scalar.activation(h_sb[:, :], hps[:, :],
                                     mybir.ActivationFunctionType.Relu)
                lps = mps.tile([P, N_CLASS], f32, tag="lps")
                nc.tensor.matmul(out=lps[:, :], lhsT=h_sb[:, :],
                                 rhs=w2_sb[:, :], start=True, stop=True)
                nc.vector.tensor_copy(L_sb[:, t * N_CLASS:(t + 1) * N_CLASS],
                                      lps[:, :])
                nc.sync.dma_start(lt_dram.ap()[t * P:(t + 1) * P, :],
                                  L_sb[:, t * N_CLASS:(t + 1) * N_CLASS])

            nc.gpsimd.collective_compute(
                "AllGather", mybir.AluOpType.bypass,
                replica_groups=[list(range(NCORES))],
                ins=[lt_dram.ap()], outs=[tbl_dram[0].ap()],
            )

            # ---------- propagation iterations ----------
            for it in range(N_ITERS):
                tbl_ap = tbl_dram[it].ap()
                for (j0, nb, soff, scnt) in chunks:
                    g = gp.tile([P, CHUNK_SLOTS * N_CLASS], f32, tag="g",
                                name=f"g_{it}_{j0}")
                    g3 = g[:, :scnt * N_CLASS].rearrange(
                        "p (s c) -> p s c", c=N_CLASS)
                    for s in range(scnt):
                        nc.gpsimd.indirect_dma_start(
                            out=g3[:, s, :],
                            out_offset=None,
                            in_=tbl_ap,
                            in_offset=bass.IndirectOffsetOnAxis(
                                ap=gidx_sb[:, soff + s:soff + s + 1], axis=0),
                        )
                    if it == 0:
                        dst = sp.tile([P, nb * N_CLASS], f32, tag="st",
                                      name=f"st_{j0}")
                        dbase = 0
                    else:
                        dst = res_sb
                        dbase = j0 * N_CLASS
                    for jj in range(nb):
                        j = j0 + jj
                        w = int(W[j])
                        m = w // 8
                        o = int(off[j]) - soff
                        blk = g[:, o * N_CLASS:(o + w) * N_CLASS].rearrange(
                            "p (s c) -> p s c", c=N_CLASS)
                        for q in range(1, m):
                            nc.vector.tensor_tensor(
                                out=blk[:, 0:8, :], in0=blk[:, 0:8, :],
                                in1=blk[:, 8 * q:8 * q + 8, :],
                                op=mybir.AluOpType.add)
                        for hw in (4, 2, 1):
                            nc.vector.tensor_tensor(
                                out=blk[:, 0:hw, :], in0=blk[:, 0:hw, :],
                                in1=blk[:, hw:2 * hw, :],
                                op=mybir.AluOpType.add)
                        dsl = slice(dbase + jj * N_CLASS,
                                    dbase + (jj + 1) * N_CLASS)
                        csl = slice(j * N_CLASS, (j + 1) * N_CLASS)
                        nc.vector.tensor_mul(dst[:, dsl], blk[:, 0, :],
                                             scale_sb[:, csl])
                        nc.vector.scalar_tensor_tensor(
                            out=dst[:, dsl], in0=L_sb[:, csl], scalar=ALPHA,
                            in1=dst[:, dsl],
                            op0=mybir.AluOpType.mult,
                            op1=mybir.AluOpType.add)
                    if it == 0:
                        nd = new_dram.ap().rearrange(
                            "(j p) c -> p j c", p=P)
                        nc.sync.dma_start(
                            nd[:, j0:j0 + nb, :],
                            dst[:, :nb * N_CLASS].rearrange(
                                "p (j c) -> p j c", c=N_CLASS))

                if it == 0:
                    nc.gpsimd.collective_compute(
                        "AllGather", mybir.AluOpType.bypass,
                        replica_groups=[list(range(NCORES))],
                        ins=[new_dram.ap()], outs=[tbl_dram[1].ap()],
                    )

            # ---------- batched log_softmax ----------
            res3 = res_sb[:, :].rearrange("p (j c) -> p j c", c=N_CLASS)
            mx = pp.tile([P, NT], f32)
            mx3 = mx[:, :].rearrange("p (j o) -> p j o", o=1)
            nc.vector.tensor_reduce(out=mx3, in_=res3,
                                    axis=mybir.AxisListType.X,
                                    op=mybir.AluOpType.max)
            shifted = pp.tile([P, FC], f32)
            sh3 = shifted[:, :].rearrange("p (j c) -> p j c", c=N_CLASS)
            nc.vector.tensor_tensor(out=sh3, in0=res3,
                                    in1=mx3.to_broadcast([P, NT, N_CLASS]),
                                    op=mybir.AluOpType.subtract)
            ex = pp.tile([P, FC], f32)
            nc.scalar.activation(ex[:, :], shifted[:, :],
                                 mybir.ActivationFunctionType.Exp)
            sm = pp.tile([P, NT], f32)
            sm3 = sm[:, :].rearrange("p (j o) -> p j o", o=1)
            nc.vector.tensor_reduce(out=sm3,
                                    in_=ex[:, :].rearrange(
                                        "p (j c) -> p j c", c=N_CLASS),
                                    axis=mybir.AxisListType.X,
                                    op=mybir.AluOpType.add)
            lg = pp.tile([P, NT], f32)
            nc.scalar.activation(lg[:, :], sm[:, :],
                                 mybir.ActivationFunctionType.Ln)
            lg3 = lg[:, :].rearrange("p (j o) -> p j o", o=1)
            fin = pp.tile([P, FC], f32)
            f3 = fin[:, :].rearrange("p (j c) -> p j c", c=N_CLASS)
            nc.vector.tensor_tensor(out=f3, in0=sh3,
                                    in1=lg3.to_broadcast([P, NT, N_CLASS]),
                                    op=mybir.AluOpType.subtract)
            oud = out_ext.rearrange("(j p) c -> p j c", p=P)
            nc.sync.dma_start(oud, f3)

    nc.compile()
    return nc


def _prepare(x, W1, W2, edge_src, edge_dst):
    x = np.asarray(x, dtype=np.float32)
    W1 = np.asarray(W1, dtype=np.float32)
    W2 = np.asarray(W2, dtype=np.float32)

    deg_full, perms, W, chunks, slots_pp, idx_arrays = _host_prep(
        edge_src, edge_dst)

    key = (tuple(int(w) for w in W), slots_pp)
    if key not in _COMPILED_CACHE:
        _COMPILED_CACHE[key] = _build_program(W, chunks, slots_pp)
    nc = _COMPILED_CACHE[key]

    w1p = np.concatenate(
        [W1[k * P:(k + 1) * P, :] for k in range(4)],
        axis=1).astype(ml_dtypes.bfloat16)          # [128, 512]
    w2b = W2.astype(ml_dtypes.bfloat16)

    in_maps = []
    for c in range(NCORES):
        perm = perms[c]
        xs = np.zeros((SHARD_PAD, N_FEAT), dtype=np.float32)
        valid = perm < SHARD
        xs[valid] = x[c * SHARD + perm[valid]]
        xT = np.ascontiguousarray(xs.T).astype(ml_dtypes.bfloat16)

        dr = deg_full[c * SHARD:(c + 1) * SHARD][perm[valid]].astype(
            np.float64)
        scale_r = np.zeros(SHARD_PAD, dtype=np.float32)
        scale_r[valid] = ((1.0 - ALPHA) /
                          np.maximum(dr, 1e-12)).astype(np.float32)
        # [rank] -> [p, j] -> repeat classes
        s_pj = scale_r.reshape(NT, P).T                      # [128, 98]
        scale16 = np.repeat(s_pj[:, :, None], N_CLASS,
                            axis=2).reshape(P, NT * N_CLASS)
        scale16 = np.ascontiguousarray(scale16, dtype=np.float32)

        in_maps.append({
            "xT": xT,
            "w1p": w1p,
            "w2": w2b,
            "scale16": scale16,
            "gidx": idx_arrays[c],
        })

    return nc, in_maps, perms


def kernel(x, W1, W2, edge_src, edge_dst):
    from concourse.bass_utils import run_bass_kernel_spmd

    nc, in_maps, perms = _prepare(x, W1, W2, edge_src, edge_dst)
    res = run_bass_kernel_spmd(nc, in_maps, list(range(NCORES)))

    out = np.empty((N_NODES, N_CLASS), dtype=np.float32)
    for c in range(NCORES):
        perm = perms[c]
        valid = perm < SHARD
        out[c * SHARD + perm[valid]] = res.results[c]["out"][valid]
    return out


# revision 5
# speedup vs baseline: 2.3828x; 2.3828x over previous
"""Trainium2 Bass kernel for 2-layer GCN + 2-step propagation + log_softmax.

v3 = baseline class-transposed ap_gather SpMM with:
  - deduplicated (src, dst) gather stream + count-weighted scan (-21% gpsimd)
  - bf16 MLP consuming host-transposed x (no PE transposes)
  - host-precomputed scale (no reciprocal chain)
  - batched log_softmax (2 ACT table loads instead of 196)
"""

import sys

sys.path.insert(0, "/opt/trn_rl_repo")

import numpy as np
import ml_dtypes

_COMPILED_CACHE = {}

N_NODES = 100000
N_FEAT = 512
HIDDEN = 128
N_CLASS = 16
N_EDGES = 3200000
ALPHA = 0.25
N_ITERS = 2

NCORES = 8
SHARD = N_NODES // NCORES          # 12500
SHARD_PAD = 12544                  # multiple of 128
N_TILES = 8                        # src tiles per shard per iteration
TILE_SRCS = SHARD_PAD // N_TILES   # 1568 srcs per tile
P = 128
NT = SHARD_PAD // P                # 98


def _host_prep(edge_src, edge_dst):
    """Per-core dedup'd gather index + count + boundary arrays."""
    es = np.asarray(edge_src).astype(np.int64)
    ed = np.asarray(edge_dst).astype(np.int64)

    deg_full = np.bincount(es, minlength=N_NODES).astype(np.float32)

    per_core = []
    core_of = es // SHARD
    grp = ed // SHARD               # dst chunk = gpsimd group
    dst_local = ed - grp * SHARD    # 0..12499, < 12544 table elems

    for c in range(NCORES):
        m = core_of == c
        s_loc = es[m] - c * SHARD
        g = grp[m]
        dl = dst_local[m]

        tile_of = s_loc // TILE_SRCS
        order = np.lexsort((dl, s_loc, g, tile_of))
        s_loc, g, dl, tile_of = s_loc[order], g[order], dl[order], tile_of[order]

        idx_tiles, cnt_tiles, bnd_tiles = [], [], []
        nidx_list, nb_list = [], []
        for t in range(N_TILES):
            tm = tile_of == t
            gt, st, dt = g[tm], s_loc[tm], dl[tm]
            s_base = t * TILE_SRCS
            group_d, group_cnt, group_bounds = [], [], []
            maxn = 0
            for j in range(NCORES):
                jm = gt == j
                sj, dj = st[jm], dt[jm]
                # dedup consecutive (src, dst) pairs (sorted by src then dst)
                if len(dj):
                    new = np.empty(len(dj), dtype=bool)
                    new[0] = True
                    new[1:] = (sj[1:] != sj[:-1]) | (dj[1:] != dj[:-1])
                    pos = np.flatnonzero(np.append(new, True))
                    cj = np.diff(pos).astype(np.float32)
                    dju = dj[new]
                    sju = sj[new]
                else:
                    cj = np.zeros(0, np.float32)
                    dju = dj
                    sju = dj
                cnt = np.bincount(sju - s_base, minlength=TILE_SRCS)
                bounds = np.concatenate([[0], np.cumsum(cnt)])
                group_d.append(dju)
                group_cnt.append(cj)
                group_bounds.append(bounds)
                maxn = max(maxn, len(dju) + 1)
            nidx = -(-max(maxn, 128) // 128) * 128
            nb = -(-(TILE_SRCS + 1) // 256) * 256
            idx_arr = np.full((P, nidx // 16), SHARD, dtype=np.int16)
            cnt_arr = np.ones((P, nidx), dtype=np.float32)
            bnd_arr = np.zeros((P, nb // 16), dtype=np.int16)
            for j in range(NCORES):
                dju = group_d[j]
                lst = np.full(nidx, SHARD, dtype=np.int64)
                lst[1: len(dju) + 1] = dju
                idx_arr[16 * j:16 * j + 16, :] = (
                    lst.reshape(nidx // 16, 16).T.astype(np.int16))
                cl = np.ones(nidx, dtype=np.float32)
                cl[1: len(dju) + 1] = group_cnt[j]
                cnt_arr[16 * j:16 * j + 16, :] = cl[None, :]
                bl = np.zeros(nb, dtype=np.int64)
                bl[: TILE_SRCS + 1] = group_bounds[j]
                bnd_arr[16 * j:16 * j + 16, :] = (
                    bl.reshape(nb // 16, 16).T.astype(np.int16))
            idx_tiles.append(idx_arr)
            cnt_tiles.append(cnt_arr)
            bnd_tiles.append(bnd_arr)
            nidx_list.append(nidx)
            nb_list.append(nb)

        per_core.append(dict(idx_tiles=idx_tiles, cnt_tiles=cnt_tiles,
                             bnd_tiles=bnd_tiles,
                             nidx_list=nidx_list, nb_list=nb_list))

    return deg_full, per_core


def _chunks_of(total, size):
    out = []
    q = 0
    while q < total:
        out.append((q, min(size, total - q)))
        q += size
    return out


def _build_program(nidx_list, nb_list):
    import concourse.bass as bass
    import concourse.tile as tile
    import concourse.mybir as mybir
    from concourse import bacc

    f32 = mybir.dt.float32
    bf16 = mybir.dt.bfloat16
    i16 = mybir.dt.int16

    nc = bacc.Bacc("TRN2", target_bir_lowering=False, debug=False,
                   num_devices=NCORES)

    # ---- I/O ----
    xT_in = nc.dram_tensor("xT", [N_FEAT, SHARD_PAD], bf16,
                           kind="ExternalInput").ap()
    w1_in = nc.dram_tensor("w1p", [P, N_FEAT], bf16,
                           kind="ExternalInput").ap()
    w2_in = nc.dram_tensor("w2", [HIDDEN, N_CLASS], bf16,
                           kind="ExternalInput").ap()
    scale_in = nc.dram_tensor("scaleT", [16, SHARD_PAD], f32,
                              kind="ExternalInput").ap()
    e16_in = nc.dram_tensor("e16", [P, 16], f32, kind="ExternalInput").ap()
    ident_in = nc.dram_tensor("ident16", [16, 16], f32,
                              kind="ExternalInput").ap()
    sum_nidx = sum(nidx_list)
    sum_nb = sum(nb_list)
    idx_in = nc.dram_tensor("gidx", [P, sum_nidx // 16], i16,
                            kind="ExternalInput").ap()
    cnt_in = nc.dram_tensor("gcnt", [P, sum_nidx], f32,
                            kind="ExternalInput").ap()
    bnd_in = nc.dram_tensor("gbnd", [P, sum_nb // 16], i16,
                            kind="ExternalInput").ap()
    out_ext = nc.dram_tensor("out", [SHARD_PAD, N_CLASS], f32,
                             kind="ExternalOutput").ap()

    # ---- internal DRAM ----
    lt_dram = nc.dram_tensor("lt_shard", [16, SHARD_PAD], f32)
    new_dram = nc.dram_tensor("newlog", [16, SHARD_PAD], f32)
    log2_dram = nc.dram_tensor("log2", [16, SHARD_PAD], f32)
    tbl_dram = [
        nc.dram_tensor(f"tbl{i}", [P * SHARD_PAD], f32, addr_space="Shared")
        for i in range(2)
    ]

    FC = NT * N_CLASS

    with tile.TileContext(nc) as tc:
        with (
            tc.tile_pool(name="persist", bufs=1) as pp,
            tc.tile_pool(name="mlp", bufs=3) as mp,
            tc.tile_pool(name="mlp_ps", bufs=2, space="PSUM") as mps,
            tc.tile_pool(name="sp1", bufs=1) as sp1,
            tc.tile_pool(name="spc", bufs=1) as spc,
            tc.tile_pool(name="sp2", bufs=2) as sp2,
            tc.tile_pool(name="chk", bufs=2) as ck,
            tc.tile_pool(name="sp_ps", bufs=2, space="PSUM") as sps,
        ):
            # ---------- constants ----------
            w1_sb = pp.tile([P, N_FEAT], bf16)
            nc.sync.dma_start(w1_sb[:, :], w1_in)
            w2_sb = pp.tile([HIDDEN, N_CLASS], bf16)
            nc.sync.dma_start(w2_sb[:, :], w2_in)
            e16_sb = pp.tile([P, 16], f32)
            nc.sync.dma_start(e16_sb[:, :], e16_in)
            ident_sb = pp.tile([16, 16], f32)
            nc.sync.dma_start(ident_sb[:, :], ident_in)
            gidx_sb = pp.tile([P, sum_nidx // 16], i16)
            nc.sync.dma_start(gidx_sb[:, :], idx_in)
            gbnd_sb = pp.tile([P, sum_nb // 16], i16)
            nc.sync.dma_start(gbnd_sb[:, :], bnd_in)

            # ---------- MLP ----------
            for t in range(NT):
                xt = mp.tile([P, N_FEAT], bf16, tag="xt")
                for k in range(4):
                    nc.sync.dma_start(
                        xt[:, k * P:(k + 1) * P],
                        xT_in[k * P:(k + 1) * P, t * P:(t + 1) * P])
                hps = mps.tile([P, P], f32, tag="hps")
                for k in range(4):
                    nc.tensor.matmul(out=hps[:, :],
                                     lhsT=w1_sb[:, k * P:(k + 1) * P],
                                     rhs=xt[:, k * P:(k + 1) * P],
                                     start=(k == 0), stop=(k == 3))
                h_sb = mp.tile([P, P], bf16, tag="h_sb")
                nc.scalar.activation(h_sb[:, :], hps[:, :],
                                     mybir.ActivationFunctionType.Relu)
                lps = mps.tile([16, P], f32, tag="lps")
                nc.tensor.matmul(out=lps[:, :], lhsT=w2_sb[:, :],
                                 rhs=h_sb[:, :], start=True, stop=True)
                ltc = mp.tile([16, P], f32, tag="ltc")
                nc.vector.tensor_copy(ltc[:, :], lps[:, :])
                nc.sync.dma_start(lt_dram.ap()[:, t * P:(t + 1) * P],
                                  ltc[:, :])

            nc.gpsimd.collective_compute(
                "AllGather", mybir.AluOpType.bypass,
                replica_groups=[list(range(NCORES))],
                ins=[lt_dram.ap()], outs=[tbl_dram[0].ap()],
            )

            # ---------- propagation iterations ----------
            for it in range(N_ITERS):
                tbl_sb = sp1.tile([P, SHARD_PAD], f32, tag="tbl",
                                  name=f"tbl_{it}")
                nc.sync.dma_start(
                    tbl_sb[:, :],
                    tbl_dram[it].ap().rearrange("(p n) -> p n", p=P))

                dst_dram = log2_dram if it == N_ITERS - 1 else new_dram

                off_i = 0
                off_b = 0
                for t in range(N_TILES):
                    nidx = nidx_list[t]
                    nb = nb_list[t]
                    gbuf = sp2.tile([P, nidx], f32, tag="gbuf",
                                    name=f"gbuf_{it}_{t}")
                    nc.gpsimd.ap_gather(
                        out_ap=gbuf[:, :].rearrange("p (n d) -> p n d", d=1),
                        in_ap=tbl_sb[:, :].rearrange("p (n d) -> p n d", d=1),
                        idxs_ap=gidx_sb[:, off_i:off_i + nidx // 16],
                        channels=P, num_elems=SHARD_PAD, d=1, num_idxs=nidx,
                    )
                    cnt_sb = spc.tile([P, nidx], f32, tag="cnt",
                                      name=f"cnt_{it}_{t}")
                    nc.sync.dma_start(cnt_sb[:, :],
                                      cnt_in[:, off_i * 16:off_i * 16 + nidx])
                    nc.vector.tensor_mul(gbuf[:, :], gbuf[:, :], cnt_sb[:, :])
                    nc.vector.tensor_tensor_scan(
                        out=gbuf[:, :],
                        data0=gbuf[:, :], data1=gbuf[:, :],
                        initial=0.0,
                        op0=mybir.AluOpType.add,
                        op1=mybir.AluOpType.bypass,
                    )
                    ext = sp2.tile([P, nb], f32, tag="ext",
                                   name=f"ext_{it}_{t}")
                    nc.gpsimd.ap_gather(
                        out_ap=ext[:, :].rearrange("p (n d) -> p n d", d=1),
                        in_ap=gbuf[:, :].rearrange("p (n d) -> p n d", d=1),
                        idxs_ap=gbnd_sb[:, off_b:off_b + nb // 16],
                        channels=P, num_elems=nidx, d=1, num_idxs=nb,
                    )
                    part = sp2.tile([P, TILE_SRCS], f32, tag="part",
                                    name=f"part_{it}_{t}")
                    nc.vector.tensor_tensor(
                        out=part[:, :],
                        in0=ext[:, 1:TILE_SRCS + 1],
                        in1=ext[:, 0:TILE_SRCS],
                        op=mybir.AluOpType.subtract,
                    )
                    for (q0, cw) in _chunks_of(TILE_SRCS, 512):
                        aps = sps.tile([16, 512], f32, tag="aps")
                        nc.tensor.matmul(out=aps[:, :cw], lhsT=e16_sb[:, :],
                                         rhs=part[:, q0:q0 + cw],
                                         start=True, stop=True)
                        col = t * TILE_SRCS + q0
                        sl = slice(col, col + cw)
                        scc = ck.tile([16, 512], f32, tag="scc")
                        nc.sync.dma_start(scc[:, :cw], scale_in[:, sl])
                        ltk = ck.tile([16, 512], f32, tag="ltk")
                        nc.sync.dma_start(ltk[:, :cw], lt_dram.ap()[:, sl])
                        tmp = ck.tile([16, 512], f32, tag="tmp")
                        nc.vector.tensor_mul(tmp[:, :cw], aps[:, :cw],
                                             scc[:, :cw])
                        outc = ck.tile([16, 512], f32, tag="outc")
                        nc.vector.scalar_tensor_tensor(
                            out=outc[:, :cw],
                            in0=ltk[:, :cw],
                            scalar=ALPHA,
                            in1=tmp[:, :cw],
                            op0=mybir.AluOpType.mult,
                            op1=mybir.AluOpType.add,
                        )
                        nc.sync.dma_start(dst_dram.ap()[:, sl], outc[:, :cw])
                    off_i += nidx // 16
                    off_b += nb // 16

                if it < N_ITERS - 1:
                    nc.gpsimd.collective_compute(
                        "AllGather", mybir.AluOpType.bypass,
                        replica_groups=[list(range(NCORES))],
                        ins=[new_dram.ap()], outs=[tbl_dram[1].ap()],
                    )

            # ---------- batched log_softmax ----------
            res_sb = pp.tile([P, FC], f32)
            for t in range(NT):
                l2c = mp.tile([16, P], f32, tag="l2c")
                nc.sync.dma_start(l2c[:, :],
                                  log2_dram.ap()[:, t * P:(t + 1) * P])
                tps = mps.tile([P, 16], f32, tag="tps")
                nc.tensor.transpose(out=tps[:, :], in_=l2c[:, :],
                                    identity=ident_sb[:, :])
                nc.vector.tensor_copy(
                    res_sb[:, t * N_CLASS:(t + 1) * N_CLASS], tps[:, :])
            res3 = res_sb[:, :].rearrange("p (j c) -> p j c", c=N_CLASS)
            mx = pp.tile([P, NT], f32)
            mx3 = mx[:, :].rearrange("p (j o) -> p j o", o=1)
            nc.vector.tensor_reduce(out=mx3, in_=res3,
                                    axis=mybir.AxisListType.X,
                                    op=mybir.AluOpType.max)
            shifted = pp.tile([P, FC], f32)
            sh3 = shifted[:, :].rearrange("p (j c) -> p j c", c=N_CLASS)
            nc.vector.tensor_tensor(out=sh3, in0=res3,
                                    in1=mx3.to_broadcast([P, NT, N_CLASS]),
                                    op=mybir.AluOpType.subtract)
            ex = pp.tile([P, FC], f32)
            nc.scalar.activation(ex[:, :], shifted[:, :],
                                 mybir.ActivationFunctionType.Exp)
            sm = pp.tile([P, NT], f32)
            sm3 = sm[:, :].rearrange("p (j o) -> p j o", o=1)
            nc.vector.tensor_reduce(out=sm3,
                                    in_=ex[:, :].rearrange(
                                        "p (j c) -> p j c", c=N_CLASS),
                                    axis=mybir.AxisListType.X,
                                    op=mybir.AluOpType.add)
            lg = pp.tile([P, NT], f32)
            nc.scalar.activation(lg[:, :], sm[:, :],
                                 mybir.ActivationFunctionType.Ln)
            lg3 = lg[:, :].rearrange("p (j o) -> p j o", o=1)
            f3 = sh3  # reuse shifted in place for the final subtract
            nc.vector.tensor_tensor(out=f3, in0=sh3,
                                    in1=lg3.to_broadcast([P, NT, N_CLASS]),
                                    op=mybir.AluOpType.subtract)
            oud = out_ext.rearrange("(j p) c -> p j c", p=P)
            nc.sync.dma_start(oud, f3)

    nc.compile()
    return nc


def _prepare(x, W1, W2, edge_src, edge_dst):
    x = np.asarray(x, dtype=np.float32)
    W1 = np.asarray(W1, dtype=np.float32)
    W2 = np.asarray(W2, dtype=np.float32)

    deg_full, per_core = _host_prep(edge_src, edge_dst)

    nidx_list = [max(pc["nidx_list"][t] for pc in per_core)
                 for t in range(N_TILES)]
    nb_list = [max(pc["nb_list"][t] for pc in per_core)
               for t in range(N_TILES)]

    key = (tuple(nidx_list), tuple(nb_list))
    if key not in _COMPILED_CACHE:
        _COMPILED_CACHE[key] = _build_program(nidx_list, nb_list)
    nc = _COMPILED_CACHE[key]

    e16 = np.tile(np.eye(16, dtype=np.float32), (8, 1))
    ident16 = np.eye(16, dtype=np.float32)
    w1p = np.concatenate(
        [W1[k * P:(k + 1) * P, :] for k in range(4)],
        axis=1).astype(ml_dtypes.bfloat16)
    w2b = W2.astype(ml_dtypes.bfloat16)

    in_maps = []
    for c in range(NCORES):
        pc = per_core[c]
        xs = np.zeros((SHARD_PAD, N_FEAT), dtype=np.float32)
        xs[:SHARD] = x[c * SHARD:(c + 1) * SHARD]
        xT = np.ascontiguousarray(xs.T).astype(ml_dtypes.bfloat16)
        scale = (1.0 - ALPHA) / np.maximum(
            np.pad(deg_full[c * SHARD:(c + 1) * SHARD],
                   (0, SHARD_PAD - SHARD), constant_values=1.0), 1e-12)
        scaleT = np.tile(scale[None, :], (16, 1)).astype(np.float32)
        idx_cat = np.concatenate(
            [np.pad(pc["idx_tiles"][t],
                    ((0, 0), (0, (nidx_list[t] - pc["nidx_list"][t]) // 16)),
                    constant_values=SHARD)
             for t in range(N_TILES)], axis=1)
        cnt_cat = np.concatenate(
            [np.pad(pc["cnt_tiles"][t],
                    ((0, 0), (0, nidx_list[t] - pc["nidx_list"][t])),
                    constant_values=1.0)
             for t in range(N_TILES)], axis=1)
        bnd_cat = np.concatenate(
            [np.pad(pc["bnd_tiles"][t],
                    ((0, 0), (0, (nb_list[t] - pc["nb_list"][t]) // 16)))
             for t in range(N_TILES)], axis=1)
        in_maps.append({
            "xT": xT,
            "w1p": w1p,
            "w2": w2b,
            "scaleT": scaleT,
            "e16": e16,
            "ident16": ident16,
            "gidx": idx_cat,
            "gcnt": cnt_cat,
            "gbnd": bnd_cat,
        })

    return nc, in_maps


def kernel(x, W1, W2, edge_src, edge_dst):
    from concourse.bass_utils import run_bass_kernel_spmd

    nc, in_maps = _prepare(x, W1, W2, edge_src, edge_dst)
    res = run_bass_kernel_spmd(nc, in_maps, list(range(NCORES)))

    out = np.empty((N_NODES, N_CLASS), dtype=np.float32)
    for c in range(NCORES):
        out[c * SHARD:(c + 1) * SHARD] = res.results[c]["out"][:SHARD]
    return out
